# revision 5
# baseline (speedup 1.0000x reference)
"""DeepFM dense-MLP kernel for 8x Trainium2 NeuronCores (Bass/Tile).

Computation (reference):
    inter = relu(x * x.sum(axis=1, keepdims=True))        # FM pairwise term
    h = x
    for i in 0..3:  h = relu(h @ W_dnn[i].T + b_dnn[i])
    out = ((h + inter) * 0.5) @ W_out.T + b_out

Strategy:
  - Data-parallel: batch B=8192 split across 8 cores (1024 rows each).
  - Feature-major activations on device: h^T [D, B_c] so every GEMM is
    psum[e, b] += W^T[d_k, e_m].T @ h^T[d_k, b]  with the weight tile
    stationary and activations streaming (N=512 per matmul).
  - Matmuls run as float32r (fp32 storage, full-rate PE streaming,
    ~16x better accuracy than bf16 at ~same speed).
  - Each core processes its 1024 columns in S=2 super-passes of 512 so
    two [D, 512] fp32 activation buffers fit in SBUF; weights stream
    from HBM once per (super-pass, layer) as contiguous 2 MB strips.
  - PSUM evictions are single fused DVE ops: (psum + bias) max 0.
    The 0.5 scale on the last-layer input is folded into layer-4's
    weights and bias on the host.
  - Row-sum s = sum_d x[b, d] computed on PE with a ones-vector matmul;
    broadcast of 0.5*s across partitions via a K=1 matmul; the FM term
    is fused into the last-layer input build:
        h5in = 0.5*h4 + relu(x^T * 0.5 s).
"""

import sys

import numpy as np

if "/opt/trn_rl_repo" not in sys.path:
    sys.path.insert(0, "/opt/trn_rl_repo")

import concourse.mybir as mybir  # noqa: E402
import concourse.tile as tile  # noqa: E402
from concourse import bacc  # noqa: E402
from concourse.bass_utils import run_bass_kernel_spmd  # noqa: E402

B, D, L = 8192, 4096, 4
NCORES = 8
BC = B // NCORES  # 1024 batch rows per core
P = 128
KK = D // P  # 32 k-tiles
MM = D // P  # 32 m-tiles
NB = 512  # matmul free dim / PSUM bank
NLAYERS = 5

# Config: float32r matmuls over fp32 storage, 2 super-passes of 512 cols.
S = 2
COLS = BC // S  # columns per super-pass
PI = COLS // NB  # inner passes per super-pass

f32 = mybir.dt.float32
f32r = mybir.dt.float32r
NPDT = np.float32


def _build():
    nc = bacc.Bacc(None, target_bir_lowering=False, debug=False)
    xt_p = nc.declare_dram_parameter("xt", [KK, P, BC], f32r, isOutput=False)
    w_p = nc.declare_dram_parameter("w", [NLAYERS, MM, P, KK * P], f32r, isOutput=False)
    bias_p = nc.declare_dram_parameter("bias", [NLAYERS, P, MM], f32, isOutput=False)
    out_p = nc.declare_dram_parameter("out", [MM, P, BC], f32, isOutput=True)

    add = mybir.AluOpType.add
    amax = mybir.AluOpType.max

    with tile.TileContext(nc) as tc:
        with (
            tc.tile_pool(name="const", bufs=1) as const,
            tc.tile_pool(name="hA", bufs=1) as hA_pool,
            tc.tile_pool(name="hB", bufs=1) as hB_pool,
            tc.tile_pool(name="wts", bufs=2) as wpool,
            tc.tile_pool(name="xst", bufs=2) as xpool,
            tc.tile_pool(name="tmp", bufs=3) as tpool,
            tc.tile_pool(name="outt", bufs=3) as opool,
            tc.tile_pool(name="sml", bufs=2) as spool,
            tc.tile_pool(name="psum", bufs=4, space="PSUM") as psum_pool,
            tc.tile_pool(name="psum_s", bufs=1, space="PSUM") as psum_s,
        ):
            bias_t = const.tile([P, NLAYERS * MM], f32)
            for l in range(NLAYERS):
                nc.sync.dma_start(out=bias_t[:, l * MM : (l + 1) * MM], in_=bias_p[l])
            ones_f = const.tile([P, 1], f32)
            nc.any.memset(ones_f[:], 1.0)
            ones_t = const.tile([P, 1], f32r)
            nc.vector.tensor_copy(out=ones_t[:], in_=ones_f[:])
            halves_f = const.tile([1, P], f32)
            nc.any.memset(halves_f[:], 0.5)
            halves_t = const.tile([1, P], f32r)
            nc.vector.tensor_copy(out=halves_t[:], in_=halves_f[:])

            for s in range(S):
                c0 = s * COLS
                A = [hA_pool.tile([P, COLS], f32r, name=f"hA{k}") for k in range(KK)]
                Bb = [hB_pool.tile([P, COLS], f32r, name=f"hB{k}") for k in range(KK)]
                for kk in range(KK):
                    nc.sync.dma_start(out=A[kk][:], in_=xt_p[kk][:, c0 : c0 + COLS])

                # sB[pi] = 0.5 * rowsum(x) broadcast over partitions
                sB = []
                for pi in range(PI):
                    csl = slice(pi * NB, (pi + 1) * NB)
                    ps_s = psum_s.tile([1, NB], f32, name="ps_s")
                    for kk in range(KK):
                        nc.tensor.matmul(
                            ps_s[:],
                            ones_t[:],
                            A[kk][:, csl],
                            start=(kk == 0),
                            stop=(kk == KK - 1),
                        )
                    s_sb = spool.tile([1, NB], f32r, name="s_sb")
                    nc.vector.tensor_copy(out=s_sb[:], in_=ps_s[:])
                    ps_b = psum_s.tile([P, NB], f32, name="ps_b")
                    nc.tensor.matmul(
                        ps_b[:], halves_t[:], s_sb[:], start=True, stop=True
                    )
                    sBt = spool.tile([P, NB], f32, name=f"sB{pi}")
                    nc.vector.tensor_copy(out=sBt[:], in_=ps_b[:])
                    sB.append(sBt)

                srcs = [A, Bb, A, Bb, Bb]
                dsts = [Bb, A, Bb, A, None]
                for l in range(NLAYERS):
                    src, dst = srcs[l], dsts[l]
                    for m in range(MM):
                        wt = wpool.tile([P, KK * P], f32r, name="wt")
                        nc.sync.dma_start(out=wt[:], in_=w_p[l, m])
                        for pi in range(PI):
                            csl = slice(pi * NB, (pi + 1) * NB)
                            ps = psum_pool.tile([P, NB], f32, name="ps")
                            for kk in range(KK):
                                nc.tensor.matmul(
                                    ps[:],
                                    wt[:, kk * P : (kk + 1) * P],
                                    src[kk][:, csl],
                                    start=(kk == 0),
                                    stop=(kk == KK - 1),
                                )
                            bsl = bias_t[:, l * MM + m : l * MM + m + 1]
                            if l < 4:
                                # dst = max(psum + bias, 0) in one DVE op
                                nc.vector.tensor_scalar(
                                    out=dst[m][:, csl],
                                    in0=ps[:],
                                    scalar1=bsl,
                                    scalar2=0.0,
                                    op0=add,
                                    op1=amax,
                                )
                            else:
                                ot = opool.tile([P, NB], f32, name="ot")
                                nc.vector.tensor_scalar_add(
                                    out=ot[:], in0=ps[:], scalar1=bsl
                                )
                                nc.sync.dma_start(
                                    out=out_p[m][:, c0 + pi * NB : c0 + (pi + 1) * NB],
                                    in_=ot[:],
                                )
                    if l == 3:
                        # h5in (into Bb) = h4half (in A) + relu(x^T * 0.5 s)
                        for kk in range(KK):
                            xst = xpool.tile([P, COLS], f32r, name="xst")
                            nc.sync.dma_start(
                                out=xst[:], in_=xt_p[kk][:, c0 : c0 + COLS]
                            )
                            for pi in range(PI):
                                csl = slice(pi * NB, (pi + 1) * NB)
                                tmp = tpool.tile([P, NB], f32, name="tmp")
                                nc.vector.tensor_mul(
                                    out=tmp[:], in0=xst[:, csl], in1=sB[pi][:]
                                )
                                nc.vector.tensor_scalar_max(
                                    out=tmp[:], in0=tmp[:], scalar1=0.0
                                )
                                nc.vector.tensor_add(
                                    out=Bb[kk][:, csl], in0=A[kk][:, csl], in1=tmp[:]
                                )
    nc.compile()
    return nc


_NC_CACHE = {}


def _get_nc():
    if "nc" not in _NC_CACHE:
        _NC_CACHE["nc"] = _build()
    return _NC_CACHE["nc"]


def _prep_weights(W_dnn, W_out, b_dnn, b_out):
    w_all = np.empty((NLAYERS, MM, P, KK * P), dtype=NPDT)
    for l in range(NLAYERS):
        W = np.asarray(W_dnn[l] if l < L else W_out, dtype=np.float32)  # [E, Din]
        if l == 3:
            W = W * 0.5  # fold the (h+inter)*0.5 into layer 3's output
        # w[l, m, p, kk*P + j] = W[m*P + j, kk*P + p]
        w_all[l] = (
            W.reshape(MM, P, KK, P)
            .transpose(0, 3, 2, 1)
            .reshape(MM, P, KK * P)
            .astype(NPDT)
        )
    b_all = np.empty((NLAYERS, P, MM), dtype=np.float32)
    for l in range(NLAYERS):
        bl = np.asarray(b_dnn[l] if l < L else b_out, dtype=np.float32)
        if l == 3:
            bl = bl * 0.5
        b_all[l] = bl.reshape(MM, P).T
    return w_all, b_all


def kernel(x, W_dnn, b_dnn, W_out, b_out):
    x = np.asarray(x, dtype=np.float32)
    w_all, b_all = _prep_weights(W_dnn, W_out, b_dnn, b_out)
    nc = _get_nc()
    in_maps = []
    for c in range(NCORES):
        xc = x[c * BC : (c + 1) * BC]  # [BC, D]
        xt = np.ascontiguousarray(xc.T).astype(NPDT).reshape(KK, P, BC)
        in_maps.append({"xt": xt, "w": w_all, "bias": b_all})
    res = run_bass_kernel_spmd(nc, in_maps, list(range(NCORES)))
    out = np.empty((B, D), dtype=np.float32)
    for c in range(NCORES):
        out[c * BC : (c + 1) * BC] = res.results[c]["out"].reshape(D, BC).T
    return out


# revision 6
# speedup vs baseline: 1.2714x; 1.2714x over previous
"""DeepFM dense-MLP kernel for 8x Trainium2 NeuronCores (Bass/Tile).

Computation (reference):
    inter = relu(x * x.sum(axis=1, keepdims=True))        # FM pairwise term
    h = x
    for i in 0..3:  h = relu(h @ W_dnn[i].T + b_dnn[i])
    out = ((h + inter) * 0.5) @ W_out.T + b_out

Strategy:
  - Data-parallel: batch B=8192 split across 8 cores (1024 rows each).
  - Feature-major activations on device: h^T [D, B_c] so every GEMM is
    psum[e, b] += W^T[d_k, e_m].T @ h^T[d_k, b]  with the weight tile
    stationary and activations streaming (N=512 per matmul).
  - dtype config: bf16 (fast weight load, weights streamed once per
    layer, both 512-col passes share each weight strip) or float32r
    (fp32 storage at full PE rate, 2 super-passes, weights streamed
    twice).
  - PSUM evictions are single fused DVE ops: (psum + bias) max 0.
    The 0.5 scale on the last-layer input is folded into layer-4's
    weights and bias on the host.
  - Row-sum s = sum_d x[b, d] computed on PE with a ones-vector matmul;
    broadcast of 0.5*s across partitions via a K=1 matmul; the FM term
    is fused into the last-layer input build:
        h5in = 0.5*h4 + relu(x^T * 0.5 s).
"""

import sys

import ml_dtypes
import numpy as np

if "/opt/trn_rl_repo" not in sys.path:
    sys.path.insert(0, "/opt/trn_rl_repo")

import concourse.mybir as mybir  # noqa: E402
import concourse.tile as tile  # noqa: E402
from concourse import bacc  # noqa: E402
from concourse.bass_utils import run_bass_kernel_spmd  # noqa: E402

B, D, L = 8192, 4096, 4
NCORES = 8
BC = B // NCORES  # 1024 batch rows per core
P = 128
KK = D // P  # 32 k-tiles
MM = D // P  # 32 m-tiles
NB = 512  # matmul free dim / PSUM bank
NLAYERS = 5

USE_F32R = False  # False -> bfloat16 matmuls

f32 = mybir.dt.float32
f32r = mybir.dt.float32r
bf16 = mybir.dt.bfloat16

if USE_F32R:
    DT = f32r
    NPDT = np.float32
    S = 2  # super-passes (weights streamed once per super-pass)
    WBUFS = 2
else:
    DT = bf16
    NPDT = ml_dtypes.bfloat16
    S = 1
    WBUFS = 4

COLS = BC // S  # columns per super-pass
PI = COLS // NB  # inner passes per super-pass


def _build():
    nc = bacc.Bacc(None, target_bir_lowering=False, debug=False)
    xt_p = nc.declare_dram_parameter("xt", [KK, P, BC], DT, isOutput=False)
    w_p = nc.declare_dram_parameter("w", [NLAYERS, MM, P, KK * P], DT, isOutput=False)
    bias_p = nc.declare_dram_parameter("bias", [NLAYERS, P, MM], f32, isOutput=False)
    out_p = nc.declare_dram_parameter("out", [MM, P, BC], f32, isOutput=True)

    add = mybir.AluOpType.add
    amax = mybir.AluOpType.max

    with tile.TileContext(nc) as tc:
        with (
            tc.tile_pool(name="const", bufs=1) as const,
            tc.tile_pool(name="hA", bufs=1) as hA_pool,
            tc.tile_pool(name="hB", bufs=1) as hB_pool,
            tc.tile_pool(name="wts", bufs=WBUFS) as wpool,
            tc.tile_pool(name="xst", bufs=2) as xpool,
            tc.tile_pool(name="tmp", bufs=3) as tpool,
            tc.tile_pool(name="outt", bufs=3) as opool,
            tc.tile_pool(name="sml", bufs=2) as spool,
            tc.tile_pool(name="psum", bufs=4, space="PSUM") as psum_pool,
            tc.tile_pool(name="psum_s", bufs=1, space="PSUM") as psum_s,
        ):
            bias_t = const.tile([P, NLAYERS * MM], f32)
            for l in range(NLAYERS):
                nc.sync.dma_start(out=bias_t[:, l * MM : (l + 1) * MM], in_=bias_p[l])
            if USE_F32R:
                # memset can't write f32r; stage via f32 + DVE copy
                ones_f = const.tile([P, 1], f32)
                nc.any.memset(ones_f[:], 1.0)
                ones_t = const.tile([P, 1], DT)
                nc.vector.tensor_copy(out=ones_t[:], in_=ones_f[:])
                halves_f = const.tile([1, P], f32)
                nc.any.memset(halves_f[:], 0.5)
                halves_t = const.tile([1, P], DT)
                nc.vector.tensor_copy(out=halves_t[:], in_=halves_f[:])
            else:
                ones_t = const.tile([P, 1], DT)
                nc.any.memset(ones_t[:], 1.0)
                halves_t = const.tile([1, P], DT)
                nc.any.memset(halves_t[:], 0.5)

            for s in range(S):
                c0 = s * COLS
                A = [hA_pool.tile([P, COLS], DT, name=f"hA{k}") for k in range(KK)]
                Bb = [hB_pool.tile([P, COLS], DT, name=f"hB{k}") for k in range(KK)]
                for kk in range(KK):
                    nc.sync.dma_start(out=A[kk][:], in_=xt_p[kk][:, c0 : c0 + COLS])

                # sB[pi] = 0.5 * rowsum(x) broadcast over partitions
                sB = []
                for pi in range(PI):
                    csl = slice(pi * NB, (pi + 1) * NB)
                    ps_s = psum_s.tile([1, NB], f32, name="ps_s")
                    for kk in range(KK):
                        nc.tensor.matmul(
                            ps_s[:],
                            ones_t[:],
                            A[kk][:, csl],
                            start=(kk == 0),
                            stop=(kk == KK - 1),
                        )
                    s_sb = spool.tile([1, NB], DT, name="s_sb")
                    nc.vector.tensor_copy(out=s_sb[:], in_=ps_s[:])
                    ps_b = psum_s.tile([P, NB], f32, name="ps_b")
                    nc.tensor.matmul(
                        ps_b[:], halves_t[:], s_sb[:], start=True, stop=True
                    )
                    sBt = spool.tile([P, NB], f32, name=f"sB{pi}")
                    nc.vector.tensor_copy(out=sBt[:], in_=ps_b[:])
                    sB.append(sBt)

                srcs = [A, Bb, A, Bb, Bb]
                dsts = [Bb, A, Bb, A, None]
                for l in range(NLAYERS):
                    src, dst = srcs[l], dsts[l]
                    for m in range(MM):
                        wt = wpool.tile([P, KK * P], DT, name="wt")
                        nc.sync.dma_start(out=wt[:], in_=w_p[l, m])
                        for pi in range(PI):
                            csl = slice(pi * NB, (pi + 1) * NB)
                            ps = psum_pool.tile([P, NB], f32, name="ps")
                            for kk in range(KK):
                                nc.tensor.matmul(
                                    ps[:],
                                    wt[:, kk * P : (kk + 1) * P],
                                    src[kk][:, csl],
                                    start=(kk == 0),
                                    stop=(kk == KK - 1),
                                )
                            bsl = bias_t[:, l * MM + m : l * MM + m + 1]
                            if l < 4:
                                # dst = max(psum + bias, 0) in one DVE op
                                nc.vector.tensor_scalar(
                                    out=dst[m][:, csl],
                                    in0=ps[:],
                                    scalar1=bsl,
                                    scalar2=0.0,
                                    op0=add,
                                    op1=amax,
                                )
                            else:
                                ot = opool.tile([P, NB], f32, name="ot")
                                nc.vector.tensor_scalar_add(
                                    out=ot[:], in0=ps[:], scalar1=bsl
                                )
                                nc.sync.dma_start(
                                    out=out_p[m][:, c0 + pi * NB : c0 + (pi + 1) * NB],
                                    in_=ot[:],
                                )
                    if l == 3:
                        # h5in (into Bb) = h4half (in A) + relu(x^T * 0.5 s)
                        for kk in range(KK):
                            xst = xpool.tile([P, COLS], DT, name="xst")
                            nc.sync.dma_start(
                                out=xst[:], in_=xt_p[kk][:, c0 : c0 + COLS]
                            )
                            for pi in range(PI):
                                csl = slice(pi * NB, (pi + 1) * NB)
                                tmp = tpool.tile([P, NB], f32, name="tmp")
                                nc.vector.tensor_mul(
                                    out=tmp[:], in0=xst[:, csl], in1=sB[pi][:]
                                )
                                nc.vector.tensor_scalar_max(
                                    out=tmp[:], in0=tmp[:], scalar1=0.0
                                )
                                nc.vector.tensor_add(
                                    out=Bb[kk][:, csl], in0=A[kk][:, csl], in1=tmp[:]
                                )
    nc.compile()
    return nc


_NC_CACHE = {}


def _get_nc():
    if "nc" not in _NC_CACHE:
        _NC_CACHE["nc"] = _build()
    return _NC_CACHE["nc"]


def _prep_weights(W_dnn, W_out, b_dnn, b_out):
    w_all = np.empty((NLAYERS, MM, P, KK * P), dtype=NPDT)
    for l in range(NLAYERS):
        W = np.asarray(W_dnn[l] if l < L else W_out, dtype=np.float32)  # [E, Din]
        if l == 3:
            W = W * 0.5  # fold the (h+inter)*0.5 into layer 3's output
        # w[l, m, p, kk*P + j] = W[m*P + j, kk*P + p]
        w_all[l] = (
            W.reshape(MM, P, KK, P)
            .transpose(0, 3, 2, 1)
            .reshape(MM, P, KK * P)
            .astype(NPDT)
        )
    b_all = np.empty((NLAYERS, P, MM), dtype=np.float32)
    for l in range(NLAYERS):
        bl = np.asarray(b_dnn[l] if l < L else b_out, dtype=np.float32)
        if l == 3:
            bl = bl * 0.5
        b_all[l] = bl.reshape(MM, P).T
    return w_all, b_all


def kernel(x, W_dnn, b_dnn, W_out, b_out):
    x = np.asarray(x, dtype=np.float32)
    w_all, b_all = _prep_weights(W_dnn, W_out, b_dnn, b_out)
    nc = _get_nc()
    in_maps = []
    for c in range(NCORES):
        xc = x[c * BC : (c + 1) * BC]  # [BC, D]
        xt = np.ascontiguousarray(xc.T).astype(NPDT).reshape(KK, P, BC)
        in_maps.append({"xt": xt, "w": w_all, "bias": b_all})
    res = run_bass_kernel_spmd(nc, in_maps, list(range(NCORES)))
    out = np.empty((B, D), dtype=np.float32)
    for c in range(NCORES):
        out[c * BC : (c + 1) * BC] = res.results[c]["out"].reshape(D, BC).T
    return out


# revision 8
# speedup vs baseline: 1.2768x; 1.0043x over previous
"""DeepFM dense-MLP kernel for 8x Trainium2 NeuronCores (Bass/Tile).

Computation (reference):
    inter = relu(x * x.sum(axis=1, keepdims=True))        # FM pairwise term
    h = x
    for i in 0..3:  h = relu(h @ W_dnn[i].T + b_dnn[i])
    out = ((h + inter) * 0.5) @ W_out.T + b_out

Strategy:
  - Data-parallel: batch B=8192 split across 8 cores (1024 rows each).
  - Feature-major activations on device: h^T [D, B_c] so every GEMM is
    psum[e, b] += W^T[d_k, e_m].T @ h^T[d_k, b]  with the weight tile
    stationary and activations streaming (N=512 per matmul).
  - dtype config: bf16 (fast weight load, weights streamed once per
    layer, both 512-col passes share each weight strip) or float32r
    (fp32 storage at full PE rate, 2 super-passes, weights streamed
    twice).
  - PSUM evictions are single fused DVE ops: (psum + bias) max 0.
    The 0.5 scale on the last-layer input is folded into layer-4's
    weights and bias on the host.
  - Row-sum s = sum_d x[b, d] computed on PE with a ones-vector matmul;
    broadcast of 0.5*s across partitions via a K=1 matmul; the FM term
    is fused into the last-layer input build:
        h5in = 0.5*h4 + relu(x^T * 0.5 s).
"""

import sys

import ml_dtypes
import numpy as np

if "/opt/trn_rl_repo" not in sys.path:
    sys.path.insert(0, "/opt/trn_rl_repo")

import concourse.mybir as mybir  # noqa: E402
import concourse.tile as tile  # noqa: E402
from concourse import bacc  # noqa: E402
from concourse.bass_utils import run_bass_kernel_spmd  # noqa: E402

B, D, L = 8192, 4096, 4
NCORES = 8
BC = B // NCORES  # 1024 batch rows per core
P = 128
KK = D // P  # 32 k-tiles
MM = D // P  # 32 m-tiles
NB = 512  # matmul free dim / PSUM bank
NLAYERS = 5

USE_F32R = False  # False -> bfloat16 matmuls

f32 = mybir.dt.float32
f32r = mybir.dt.float32r
bf16 = mybir.dt.bfloat16

if USE_F32R:
    DT = f32r
    NPDT = np.float32
    S = 2  # super-passes (weights streamed once per super-pass)
    WBUFS = 2
else:
    DT = bf16
    NPDT = ml_dtypes.bfloat16
    S = 1
    WBUFS = 6

COLS = BC // S  # columns per super-pass
PI = COLS // NB  # inner passes per super-pass


def _build():
    nc = bacc.Bacc(None, target_bir_lowering=False, debug=False)
    xt_p = nc.declare_dram_parameter("xt", [KK, P, BC], DT, isOutput=False)
    w_p = nc.declare_dram_parameter("w", [NLAYERS, MM, P, KK * P], DT, isOutput=False)
    bias_p = nc.declare_dram_parameter("bias", [NLAYERS, P, MM], f32, isOutput=False)
    out_p = nc.declare_dram_parameter("out", [MM, P, BC], f32, isOutput=True)

    add = mybir.AluOpType.add
    amax = mybir.AluOpType.max

    with tile.TileContext(nc) as tc:
        with (
            tc.tile_pool(name="const", bufs=1) as const,
            tc.tile_pool(name="hA", bufs=1) as hA_pool,
            tc.tile_pool(name="hB", bufs=1) as hB_pool,
            tc.tile_pool(name="wts", bufs=WBUFS) as wpool,
            tc.tile_pool(name="xst", bufs=2) as xpool,
            tc.tile_pool(name="tmp", bufs=3) as tpool,
            tc.tile_pool(name="outt", bufs=3) as opool,
            tc.tile_pool(name="sml", bufs=2) as spool,
            tc.tile_pool(name="psum", bufs=4, space="PSUM") as psum_pool,
            tc.tile_pool(name="psum_s", bufs=1, space="PSUM") as psum_s,
        ):
            bias_t = const.tile([P, NLAYERS * MM], f32)
            for l in range(NLAYERS):
                nc.sync.dma_start(out=bias_t[:, l * MM : (l + 1) * MM], in_=bias_p[l])
            if USE_F32R:
                # memset can't write f32r; stage via f32 + DVE copy
                ones_f = const.tile([P, 1], f32)
                nc.any.memset(ones_f[:], 1.0)
                ones_t = const.tile([P, 1], DT)
                nc.vector.tensor_copy(out=ones_t[:], in_=ones_f[:])
                halves_f = const.tile([1, P], f32)
                nc.any.memset(halves_f[:], 0.5)
                halves_t = const.tile([1, P], DT)
                nc.vector.tensor_copy(out=halves_t[:], in_=halves_f[:])
            else:
                ones_t = const.tile([P, 1], DT)
                nc.any.memset(ones_t[:], 1.0)
                halves_t = const.tile([1, P], DT)
                nc.any.memset(halves_t[:], 0.5)

            for s in range(S):
                c0 = s * COLS
                A = [hA_pool.tile([P, COLS], DT, name=f"hA{k}") for k in range(KK)]
                Bb = [hB_pool.tile([P, COLS], DT, name=f"hB{k}") for k in range(KK)]
                for kk in range(KK):
                    nc.sync.dma_start(out=A[kk][:], in_=xt_p[kk][:, c0 : c0 + COLS])

                # sB[pi] = 0.5 * rowsum(x) broadcast over partitions
                sB = []
                for pi in range(PI):
                    csl = slice(pi * NB, (pi + 1) * NB)
                    ps_s = psum_s.tile([1, NB], f32, name="ps_s")
                    for kk in range(KK):
                        nc.tensor.matmul(
                            ps_s[:],
                            ones_t[:],
                            A[kk][:, csl],
                            start=(kk == 0),
                            stop=(kk == KK - 1),
                        )
                    s_sb = spool.tile([1, NB], DT, name="s_sb")
                    nc.vector.tensor_copy(out=s_sb[:], in_=ps_s[:])
                    ps_b = psum_s.tile([P, NB], f32, name="ps_b")
                    nc.tensor.matmul(
                        ps_b[:], halves_t[:], s_sb[:], start=True, stop=True
                    )
                    sBt = spool.tile([P, NB], f32, name=f"sB{pi}")
                    nc.vector.tensor_copy(out=sBt[:], in_=ps_b[:])
                    sB.append(sBt)

                srcs = [A, Bb, A, Bb, Bb]
                dsts = [Bb, A, Bb, A, None]
                for l in range(NLAYERS):
                    src, dst = srcs[l], dsts[l]
                    for m in range(MM):
                        wt = wpool.tile([P, KK * P], DT, name="wt")
                        nc.sync.dma_start(out=wt[:], in_=w_p[l, m])
                        for pi in range(PI):
                            csl = slice(pi * NB, (pi + 1) * NB)
                            ps = psum_pool.tile([P, NB], f32, name="ps")
                            for kk in range(KK):
                                nc.tensor.matmul(
                                    ps[:],
                                    wt[:, kk * P : (kk + 1) * P],
                                    src[kk][:, csl],
                                    start=(kk == 0),
                                    stop=(kk == KK - 1),
                                )
                            bsl = bias_t[:, l * MM + m : l * MM + m + 1]
                            if l < 4:
                                if USE_F32R:
                                    # dst = max(psum + bias, 0) in one DVE op
                                    # (ACT can't produce f32r outputs)
                                    nc.vector.tensor_scalar(
                                        out=dst[m][:, csl],
                                        in0=ps[:],
                                        scalar1=bsl,
                                        scalar2=0.0,
                                        op0=add,
                                        op1=amax,
                                    )
                                else:
                                    # keep DVE free for the FM-term build;
                                    # ScalarE is otherwise idle
                                    nc.scalar.activation(
                                        dst[m][:, csl],
                                        ps[:],
                                        mybir.ActivationFunctionType.Relu,
                                        bias=bsl,
                                    )
                            else:
                                ot = opool.tile([P, NB], f32, name="ot")
                                nc.vector.tensor_scalar_add(
                                    out=ot[:], in0=ps[:], scalar1=bsl
                                )
                                nc.sync.dma_start(
                                    out=out_p[m][:, c0 + pi * NB : c0 + (pi + 1) * NB],
                                    in_=ot[:],
                                )
                    if l == 3:
                        # h5in (into Bb) = h4half (in A) + relu(x^T * 0.5 s)
                        for kk in range(KK):
                            xst = xpool.tile([P, COLS], DT, name="xst")
                            nc.sync.dma_start(
                                out=xst[:], in_=xt_p[kk][:, c0 : c0 + COLS]
                            )
                            for pi in range(PI):
                                csl = slice(pi * NB, (pi + 1) * NB)
                                tmp = tpool.tile([P, NB], f32, name="tmp")
                                nc.vector.tensor_mul(
                                    out=tmp[:], in0=xst[:, csl], in1=sB[pi][:]
                                )
                                nc.vector.tensor_scalar_max(
                                    out=tmp[:], in0=tmp[:], scalar1=0.0
                                )
                                nc.vector.tensor_add(
                                    out=Bb[kk][:, csl], in0=A[kk][:, csl], in1=tmp[:]
                                )
    nc.compile()
    return nc


_NC_CACHE = {}


def _get_nc():
    if "nc" not in _NC_CACHE:
        _NC_CACHE["nc"] = _build()
    return _NC_CACHE["nc"]


def _prep_weights(W_dnn, W_out, b_dnn, b_out):
    w_all = np.empty((NLAYERS, MM, P, KK * P), dtype=NPDT)
    for l in range(NLAYERS):
        W = np.asarray(W_dnn[l] if l < L else W_out, dtype=np.float32)  # [E, Din]
        if l == 3:
            W = W * 0.5  # fold the (h+inter)*0.5 into layer 3's output
        # w[l, m, p, kk*P + j] = W[m*P + j, kk*P + p]
        w_all[l] = (
            W.reshape(MM, P, KK, P)
            .transpose(0, 3, 2, 1)
            .reshape(MM, P, KK * P)
            .astype(NPDT)
        )
    b_all = np.empty((NLAYERS, P, MM), dtype=np.float32)
    for l in range(NLAYERS):
        bl = np.asarray(b_dnn[l] if l < L else b_out, dtype=np.float32)
        if l == 3:
            bl = bl * 0.5
        b_all[l] = bl.reshape(MM, P).T
    return w_all, b_all


def kernel(x, W_dnn, b_dnn, W_out, b_out):
    x = np.asarray(x, dtype=np.float32)
    w_all, b_all = _prep_weights(W_dnn, W_out, b_dnn, b_out)
    nc = _get_nc()
    in_maps = []
    for c in range(NCORES):
        xc = x[c * BC : (c + 1) * BC]  # [BC, D]
        xt = np.ascontiguousarray(xc.T).astype(NPDT).reshape(KK, P, BC)
        in_maps.append({"xt": xt, "w": w_all, "bias": b_all})
    res = run_bass_kernel_spmd(nc, in_maps, list(range(NCORES)))
    out = np.empty((B, D), dtype=np.float32)
    for c in range(NCORES):
        out[c * BC : (c + 1) * BC] = res.results[c]["out"].reshape(D, BC).T
    return out


# revision 10
# speedup vs baseline: 1.3103x; 1.0262x over previous
"""DeepFM dense-MLP kernel for 8x Trainium2 NeuronCores (Bass/Tile).

Computation (reference):
    inter = relu(x * x.sum(axis=1, keepdims=True))        # FM pairwise term
    h = x
    for i in 0..3:  h = relu(h @ W_dnn[i].T + b_dnn[i])
    out = ((h + inter) * 0.5) @ W_out.T + b_out

Strategy:
  - Data-parallel: batch B=8192 split across 8 cores (1024 rows each).
  - Feature-major activations on device: h^T [D, B_c] so every GEMM is
    psum[e, b] += W^T[d_k, e_m].T @ h^T[d_k, b]  with the weight tile
    stationary and activations streaming (N=512 per matmul).
  - dtype config: bf16 (fast weight load, weights streamed once per
    layer, both 512-col passes share each weight strip) or float32r
    (fp32 storage at full PE rate, 2 super-passes, weights streamed
    twice).
  - PSUM evictions are single fused DVE ops: (psum + bias) max 0.
    The 0.5 scale on the last-layer input is folded into layer-4's
    weights and bias on the host.
  - Row-sum s = sum_d x[b, d] computed on PE with a ones-vector matmul;
    broadcast of 0.5*s across partitions via a K=1 matmul; the FM term
    is fused into the last-layer input build:
        h5in = 0.5*h4 + relu(x^T * 0.5 s).
"""

import sys

import ml_dtypes
import numpy as np

if "/opt/trn_rl_repo" not in sys.path:
    sys.path.insert(0, "/opt/trn_rl_repo")

import concourse.mybir as mybir  # noqa: E402
import concourse.tile as tile  # noqa: E402
from concourse import bacc  # noqa: E402
from concourse.bass_utils import run_bass_kernel_spmd  # noqa: E402

B, D, L = 8192, 4096, 4
NCORES = 8
BC = B // NCORES  # 1024 batch rows per core
P = 128
KK = D // P  # 32 k-tiles
MM = D // P  # 32 m-tiles
NB = 512  # matmul free dim / PSUM bank
NLAYERS = 5

USE_F32R = False  # False -> bfloat16 matmuls

f32 = mybir.dt.float32
f32r = mybir.dt.float32r
bf16 = mybir.dt.bfloat16

if USE_F32R:
    DT = f32r
    NPDT = np.float32
    S = 2  # super-passes (weights streamed once per super-pass)
    WBUFS = 2
else:
    DT = bf16
    NPDT = ml_dtypes.bfloat16
    S = 1
    WBUFS = 6

COLS = BC // S  # columns per super-pass
PI = COLS // NB  # inner passes per super-pass


def _build():
    nc = bacc.Bacc(None, target_bir_lowering=False, debug=False)
    xt_p = nc.declare_dram_parameter("xt", [KK, P, BC], DT, isOutput=False)
    w_p = nc.declare_dram_parameter("w", [NLAYERS, MM, P, KK * P], DT, isOutput=False)
    bias_p = nc.declare_dram_parameter("bias", [NLAYERS, P, MM], f32, isOutput=False)
    out_p = nc.declare_dram_parameter("out", [MM, P, BC], f32, isOutput=True)

    add = mybir.AluOpType.add
    amax = mybir.AluOpType.max

    with tile.TileContext(nc) as tc:
        with (
            tc.tile_pool(name="const", bufs=1) as const,
            tc.tile_pool(name="hA", bufs=1) as hA_pool,
            tc.tile_pool(name="hB", bufs=1) as hB_pool,
            tc.tile_pool(name="wts", bufs=WBUFS) as wpool,
            tc.tile_pool(name="xst", bufs=2) as xpool,
            tc.tile_pool(name="tmp", bufs=3) as tpool,
            tc.tile_pool(name="outt", bufs=3) as opool,
            tc.tile_pool(name="sml", bufs=2) as spool,
            tc.tile_pool(name="psum", bufs=4, space="PSUM") as psum_pool,
            tc.tile_pool(name="psum_s", bufs=1, space="PSUM") as psum_s,
        ):
            bias_t = const.tile([P, NLAYERS * MM], f32)
            for l in range(NLAYERS):
                nc.sync.dma_start(out=bias_t[:, l * MM : (l + 1) * MM], in_=bias_p[l])
            if USE_F32R:
                # memset can't write f32r; stage via f32 + DVE copy
                ones_f = const.tile([P, 1], f32)
                nc.any.memset(ones_f[:], 1.0)
                ones_t = const.tile([P, 1], DT)
                nc.vector.tensor_copy(out=ones_t[:], in_=ones_f[:])
                halves_f = const.tile([1, P], f32)
                nc.any.memset(halves_f[:], 0.5)
                halves_t = const.tile([1, P], DT)
                nc.vector.tensor_copy(out=halves_t[:], in_=halves_f[:])
            else:
                ones_t = const.tile([P, 1], DT)
                nc.any.memset(ones_t[:], 1.0)
                halves_t = const.tile([1, P], DT)
                nc.any.memset(halves_t[:], 0.5)

            for s in range(S):
                c0 = s * COLS
                A = [hA_pool.tile([P, COLS], DT, name=f"hA{k}") for k in range(KK)]
                Bb = [hB_pool.tile([P, COLS], DT, name=f"hB{k}") for k in range(KK)]
                for kk in range(KK):
                    nc.sync.dma_start(out=A[kk][:], in_=xt_p[kk][:, c0 : c0 + COLS])

                # sB[pi] = 0.5 * rowsum(x) broadcast over partitions
                sB = []
                for pi in range(PI):
                    csl = slice(pi * NB, (pi + 1) * NB)
                    ps_s = psum_s.tile([1, NB], f32, name="ps_s")
                    for kk in range(KK):
                        nc.tensor.matmul(
                            ps_s[:],
                            ones_t[:],
                            A[kk][:, csl],
                            start=(kk == 0),
                            stop=(kk == KK - 1),
                        )
                    s_sb = spool.tile([1, NB], DT, name="s_sb")
                    nc.vector.tensor_copy(out=s_sb[:], in_=ps_s[:])
                    ps_b = psum_s.tile([P, NB], f32, name="ps_b")
                    nc.tensor.matmul(
                        ps_b[:], halves_t[:], s_sb[:], start=True, stop=True
                    )
                    sBt = spool.tile([P, NB], f32, name=f"sB{pi}")
                    nc.vector.tensor_copy(out=sBt[:], in_=ps_b[:])
                    sB.append(sBt)

                # layer chain A->B->A->B->A; the FM term is added IN PLACE
                # into A (h4half) right after each layer-3 m-tile evicts, so
                # it fully overlaps layer 3 instead of serializing before
                # layer 4 (no WAR against layer-3's reads of Bb).
                srcs = [A, Bb, A, Bb, A]
                dsts = [Bb, A, Bb, A, None]
                for l in range(NLAYERS):
                    src, dst = srcs[l], dsts[l]
                    for m in range(MM):
                        wt = wpool.tile([P, KK * P], DT, name="wt")
                        nc.sync.dma_start(out=wt[:], in_=w_p[l, m])
                        for pi in range(PI):
                            csl = slice(pi * NB, (pi + 1) * NB)
                            ps = psum_pool.tile([P, NB], f32, name="ps")
                            for kk in range(KK):
                                nc.tensor.matmul(
                                    ps[:],
                                    wt[:, kk * P : (kk + 1) * P],
                                    src[kk][:, csl],
                                    start=(kk == 0),
                                    stop=(kk == KK - 1),
                                )
                            bsl = bias_t[:, l * MM + m : l * MM + m + 1]
                            if l < 4:
                                if USE_F32R:
                                    # dst = max(psum + bias, 0) in one DVE op
                                    # (ACT can't produce f32r outputs)
                                    nc.vector.tensor_scalar(
                                        out=dst[m][:, csl],
                                        in0=ps[:],
                                        scalar1=bsl,
                                        scalar2=0.0,
                                        op0=add,
                                        op1=amax,
                                    )
                                else:
                                    # keep DVE free for the FM-term build;
                                    # ScalarE is otherwise idle
                                    nc.scalar.activation(
                                        dst[m][:, csl],
                                        ps[:],
                                        mybir.ActivationFunctionType.Relu,
                                        bias=bsl,
                                    )
                            else:
                                ot = opool.tile([P, NB], f32, name="ot")
                                nc.vector.tensor_scalar_add(
                                    out=ot[:], in0=ps[:], scalar1=bsl
                                )
                                nc.sync.dma_start(
                                    out=out_p[m][:, c0 + pi * NB : c0 + (pi + 1) * NB],
                                    in_=ot[:],
                                )
                    if l == 3:
                        # A[kk] += relu(x^T * 0.5 s)   (h5in build, in place)
                        for kk in range(KK):
                            xst = xpool.tile([P, COLS], DT, name="xst")
                            nc.sync.dma_start(
                                out=xst[:], in_=xt_p[kk][:, c0 : c0 + COLS]
                            )
                            for pi in range(PI):
                                csl = slice(pi * NB, (pi + 1) * NB)
                                tmp = tpool.tile([P, NB], f32, name="tmp")
                                nc.vector.tensor_mul(
                                    out=tmp[:], in0=xst[:, csl], in1=sB[pi][:]
                                )
                                # A = max(tmp, 0) + A in one fused DVE op
                                nc.vector.scalar_tensor_tensor(
                                    out=A[kk][:, csl],
                                    in0=tmp[:],
                                    scalar=0.0,
                                    in1=A[kk][:, csl],
                                    op0=amax,
                                    op1=add,
                                )
    nc.compile()
    return nc


_NC_CACHE = {}


def _get_nc():
    if "nc" not in _NC_CACHE:
        _NC_CACHE["nc"] = _build()
    return _NC_CACHE["nc"]


def _prep_weights(W_dnn, W_out, b_dnn, b_out):
    w_all = np.empty((NLAYERS, MM, P, KK * P), dtype=NPDT)
    for l in range(NLAYERS):
        W = np.asarray(W_dnn[l] if l < L else W_out, dtype=np.float32)  # [E, Din]
        if l == 3:
            W = W * 0.5  # fold the (h+inter)*0.5 into layer 3's output
        # w[l, m, p, kk*P + j] = W[m*P + j, kk*P + p]
        w_all[l] = (
            W.reshape(MM, P, KK, P)
            .transpose(0, 3, 2, 1)
            .reshape(MM, P, KK * P)
            .astype(NPDT)
        )
    b_all = np.empty((NLAYERS, P, MM), dtype=np.float32)
    for l in range(NLAYERS):
        bl = np.asarray(b_dnn[l] if l < L else b_out, dtype=np.float32)
        if l == 3:
            bl = bl * 0.5
        b_all[l] = bl.reshape(MM, P).T
    return w_all, b_all


def kernel(x, W_dnn, b_dnn, W_out, b_out):
    x = np.asarray(x, dtype=np.float32)
    w_all, b_all = _prep_weights(W_dnn, W_out, b_dnn, b_out)
    nc = _get_nc()
    in_maps = []
    for c in range(NCORES):
        xc = x[c * BC : (c + 1) * BC]  # [BC, D]
        xt = np.ascontiguousarray(xc.T).astype(NPDT).reshape(KK, P, BC)
        in_maps.append({"xt": xt, "w": w_all, "bias": b_all})
    res = run_bass_kernel_spmd(nc, in_maps, list(range(NCORES)))
    out = np.empty((B, D), dtype=np.float32)
    for c in range(NCORES):
        out[c * BC : (c + 1) * BC] = res.results[c]["out"].reshape(D, BC).T
    return out


# revision 13
# speedup vs baseline: 1.3125x; 1.0017x over previous
"""DeepFM dense-MLP kernel for 8x Trainium2 NeuronCores (Bass/Tile).

Computation (reference):
    inter = relu(x * x.sum(axis=1, keepdims=True))        # FM pairwise term
    h = x
    for i in 0..3:  h = relu(h @ W_dnn[i].T + b_dnn[i])
    out = ((h + inter) * 0.5) @ W_out.T + b_out

Strategy:
  - Data-parallel: batch B=8192 split across 8 cores (1024 rows each).
  - Feature-major activations on device: h^T [D, B_c] so every GEMM is
    psum[e, b] += W^T[d_k, e_m].T @ h^T[d_k, b]  with the weight tile
    stationary and activations streaming (N=512 per matmul).
  - dtype config: bf16 (fast weight load, weights streamed once per
    layer, both 512-col passes share each weight strip) or float32r
    (fp32 storage at full PE rate, 2 super-passes, weights streamed
    twice).
  - PSUM evictions are single fused DVE ops: (psum + bias) max 0.
    The 0.5 scale on the last-layer input is folded into layer-4's
    weights and bias on the host.
  - Row-sum s = sum_d x[b, d] computed on PE with a ones-vector matmul;
    broadcast of 0.5*s across partitions via a K=1 matmul; the FM term
    is fused into the last-layer input build:
        h5in = 0.5*h4 + relu(x^T * 0.5 s).
"""

import sys

import ml_dtypes
import numpy as np

if "/opt/trn_rl_repo" not in sys.path:
    sys.path.insert(0, "/opt/trn_rl_repo")

import concourse.mybir as mybir  # noqa: E402
import concourse.tile as tile  # noqa: E402
from concourse import bacc  # noqa: E402
from concourse.bass_utils import run_bass_kernel_spmd  # noqa: E402

B, D, L = 8192, 4096, 4
NCORES = 8
BC = B // NCORES  # 1024 batch rows per core
P = 128
KK = D // P  # 32 k-tiles
MM = D // P  # 32 m-tiles
NB = 512  # matmul free dim / PSUM bank
NLAYERS = 5

USE_F32R = False  # False -> bfloat16 matmuls

f32 = mybir.dt.float32
f32r = mybir.dt.float32r
bf16 = mybir.dt.bfloat16

if USE_F32R:
    DT = f32r
    NPDT = np.float32
    S = 2  # super-passes (weights streamed once per super-pass)
    WBUFS = 2
else:
    DT = bf16
    NPDT = ml_dtypes.bfloat16
    S = 1
    WBUFS = 5

COLS = BC // S  # columns per super-pass
PI = COLS // NB  # inner passes per super-pass


def _build():
    nc = bacc.Bacc(None, target_bir_lowering=False, debug=False)
    xt_p = nc.declare_dram_parameter("xt", [KK, P, BC], DT, isOutput=False)
    w_p = nc.declare_dram_parameter("w", [NLAYERS, MM, P, KK * P], DT, isOutput=False)
    bias_p = nc.declare_dram_parameter("bias", [NLAYERS, P, MM], f32, isOutput=False)
    out_p = nc.declare_dram_parameter("out", [MM, P, BC], f32, isOutput=True)

    add = mybir.AluOpType.add
    amax = mybir.AluOpType.max

    with tile.TileContext(nc) as tc:
        with (
            tc.tile_pool(name="const", bufs=1) as const,
            tc.tile_pool(name="hA", bufs=1) as hA_pool,
            tc.tile_pool(name="hB", bufs=1) as hB_pool,
            tc.tile_pool(name="wts", bufs=WBUFS) as wpool,
            tc.tile_pool(name="xst", bufs=2) as xpool,
            tc.tile_pool(name="tmp", bufs=3) as tpool,
            tc.tile_pool(name="outt", bufs=3) as opool,
            tc.tile_pool(name="sml", bufs=2) as spool,
            tc.tile_pool(name="psum", bufs=4, space="PSUM") as psum_pool,
            tc.tile_pool(name="psum_s", bufs=1, space="PSUM") as psum_s,
        ):
            bias_t = const.tile([P, NLAYERS * MM], f32)
            for l in range(NLAYERS):
                nc.sync.dma_start(out=bias_t[:, l * MM : (l + 1) * MM], in_=bias_p[l])
            if USE_F32R:
                # memset can't write f32r; stage via f32 + DVE copy
                ones_f = const.tile([P, 1], f32)
                nc.any.memset(ones_f[:], 1.0)
                ones_t = const.tile([P, 1], DT)
                nc.vector.tensor_copy(out=ones_t[:], in_=ones_f[:])
                halves_f = const.tile([1, P], f32)
                nc.any.memset(halves_f[:], 0.5)
                halves_t = const.tile([1, P], DT)
                nc.vector.tensor_copy(out=halves_t[:], in_=halves_f[:])
            else:
                ones_t = const.tile([P, 1], DT)
                nc.any.memset(ones_t[:], 1.0)
                halves_t = const.tile([1, P], DT)
                nc.any.memset(halves_t[:], 0.5)

            for s in range(S):
                c0 = s * COLS
                A = [hA_pool.tile([P, COLS], DT, name=f"hA{k}") for k in range(KK)]
                Bb = [hB_pool.tile([P, COLS], DT, name=f"hB{k}") for k in range(KK)]
                for kk in range(KK):
                    nc.sync.dma_start(out=A[kk][:], in_=xt_p[kk][:, c0 : c0 + COLS])

                # sB[pi] = 0.5 * rowsum(x) broadcast over partitions.
                # Partial-sum the 32 k-tiles elementwise on DVE first (PE is
                # busy-critical; this costs it only 2 matmuls per pass instead
                # of 33): acc[p, b] = sum_kk A[kk][p, b], then one M=1
                # ones-matmul finishes the cross-partition sum and one K=1
                # matmul broadcasts 0.5*s to all partitions.
                accT = const.tile([P, COLS], f32, name=f"accT{s}")
                nc.vector.tensor_add(out=accT[:], in0=A[0][:], in1=A[1][:])
                for kk in range(2, KK):
                    nc.vector.tensor_add(out=accT[:], in0=accT[:], in1=A[kk][:])
                accB = const.tile([P, COLS], DT, name=f"accB{s}")
                nc.vector.tensor_copy(out=accB[:], in_=accT[:])
                sB = []
                for pi in range(PI):
                    csl = slice(pi * NB, (pi + 1) * NB)
                    ps_s = psum_s.tile([1, NB], f32, name="ps_s")
                    nc.tensor.matmul(
                        ps_s[:], ones_t[:], accB[:, csl], start=True, stop=True
                    )
                    s_sb = spool.tile([1, NB], DT, name="s_sb")
                    nc.vector.tensor_copy(out=s_sb[:], in_=ps_s[:])
                    ps_b = psum_s.tile([P, NB], f32, name="ps_b")
                    nc.tensor.matmul(
                        ps_b[:], halves_t[:], s_sb[:], start=True, stop=True
                    )
                    sBt = spool.tile([P, NB], f32, name=f"sB{pi}")
                    nc.vector.tensor_copy(out=sBt[:], in_=ps_b[:])
                    sB.append(sBt)

                # layer chain A->B->A->B->A; the FM term is added IN PLACE
                # into A (h4half) right after each layer-3 m-tile evicts, so
                # it fully overlaps layer 3 instead of serializing before
                # layer 4 (no WAR against layer-3's reads of Bb).
                srcs = [A, Bb, A, Bb, A]
                dsts = [Bb, A, Bb, A, None]
                for l in range(NLAYERS):
                    src, dst = srcs[l], dsts[l]
                    for m in range(MM):
                        wt = wpool.tile([P, KK * P], DT, name="wt")
                        nc.sync.dma_start(out=wt[:], in_=w_p[l, m])
                        for pi in range(PI):
                            csl = slice(pi * NB, (pi + 1) * NB)
                            ps = psum_pool.tile([P, NB], f32, name="ps")
                            for kk in range(KK):
                                nc.tensor.matmul(
                                    ps[:],
                                    wt[:, kk * P : (kk + 1) * P],
                                    src[kk][:, csl],
                                    start=(kk == 0),
                                    stop=(kk == KK - 1),
                                )
                            bsl = bias_t[:, l * MM + m : l * MM + m + 1]
                            if l < 4:
                                if USE_F32R:
                                    # dst = max(psum + bias, 0) in one DVE op
                                    # (ACT can't produce f32r outputs)
                                    nc.vector.tensor_scalar(
                                        out=dst[m][:, csl],
                                        in0=ps[:],
                                        scalar1=bsl,
                                        scalar2=0.0,
                                        op0=add,
                                        op1=amax,
                                    )
                                else:
                                    # keep DVE free for the FM-term build;
                                    # ScalarE is otherwise idle
                                    nc.scalar.activation(
                                        dst[m][:, csl],
                                        ps[:],
                                        mybir.ActivationFunctionType.Relu,
                                        bias=bsl,
                                    )
                            else:
                                ot = opool.tile([P, NB], f32, name="ot")
                                nc.vector.tensor_scalar_add(
                                    out=ot[:], in0=ps[:], scalar1=bsl
                                )
                                nc.sync.dma_start(
                                    out=out_p[m][:, c0 + pi * NB : c0 + (pi + 1) * NB],
                                    in_=ot[:],
                                )
                    if l == 3:
                        # A[kk] += relu(x^T * 0.5 s)   (h5in build, in place)
                        for kk in range(KK):
                            xst = xpool.tile([P, COLS], DT, name="xst")
                            nc.sync.dma_start(
                                out=xst[:], in_=xt_p[kk][:, c0 : c0 + COLS]
                            )
                            for pi in range(PI):
                                csl = slice(pi * NB, (pi + 1) * NB)
                                tmp = tpool.tile([P, NB], f32, name="tmp")
                                nc.vector.tensor_mul(
                                    out=tmp[:], in0=xst[:, csl], in1=sB[pi][:]
                                )
                                # A = max(tmp, 0) + A in one fused DVE op
                                nc.vector.scalar_tensor_tensor(
                                    out=A[kk][:, csl],
                                    in0=tmp[:],
                                    scalar=0.0,
                                    in1=A[kk][:, csl],
                                    op0=amax,
                                    op1=add,
                                )
    nc.compile()
    return nc


_NC_CACHE = {}


def _get_nc():
    if "nc" not in _NC_CACHE:
        _NC_CACHE["nc"] = _build()
    return _NC_CACHE["nc"]


def _prep_weights(W_dnn, W_out, b_dnn, b_out):
    w_all = np.empty((NLAYERS, MM, P, KK * P), dtype=NPDT)
    for l in range(NLAYERS):
        W = np.asarray(W_dnn[l] if l < L else W_out, dtype=np.float32)  # [E, Din]
        if l == 3:
            W = W * 0.5  # fold the (h+inter)*0.5 into layer 3's output
        # w[l, m, p, kk*P + j] = W[m*P + j, kk*P + p]
        w_all[l] = (
            W.reshape(MM, P, KK, P)
            .transpose(0, 3, 2, 1)
            .reshape(MM, P, KK * P)
            .astype(NPDT)
        )
    b_all = np.empty((NLAYERS, P, MM), dtype=np.float32)
    for l in range(NLAYERS):
        bl = np.asarray(b_dnn[l] if l < L else b_out, dtype=np.float32)
        if l == 3:
            bl = bl * 0.5
        b_all[l] = bl.reshape(MM, P).T
    return w_all, b_all


def kernel(x, W_dnn, b_dnn, W_out, b_out):
    x = np.asarray(x, dtype=np.float32)
    w_all, b_all = _prep_weights(W_dnn, W_out, b_dnn, b_out)
    nc = _get_nc()
    in_maps = []
    for c in range(NCORES):
        xc = x[c * BC : (c + 1) * BC]  # [BC, D]
        xt = np.ascontiguousarray(xc.T).astype(NPDT).reshape(KK, P, BC)
        in_maps.append({"xt": xt, "w": w_all, "bias": b_all})
    res = run_bass_kernel_spmd(nc, in_maps, list(range(NCORES)))
    out = np.empty((B, D), dtype=np.float32)
    for c in range(NCORES):
        out[c * BC : (c + 1) * BC] = res.results[c]["out"].reshape(D, BC).T
    return out
